# revision 21
# baseline (speedup 1.0000x reference)
"""Distributed causal multi-head attention for 8 TRN2 NeuronCores.

Problem: x[2, 2048, 1024], 16 heads x 64 dim, causal softmax attention,
output projection. Sharding: tensor-parallel over (batch, head-group):
core c handles batch c//4 and heads [4*(c%4), 4*(c%4)+4). Each core
computes its 4 heads' attention plus the partial output projection
(sum over its heads); the host sums the 4 partials per batch.

On-device layout strategy (no transposes anywhere on device):
  - host feeds xT = x[b].T               [D=1024, S=2048]
  - wq/wk/wv = W[heads] host-packed as [128, 8*256] (partition-major,
    d-chunk-major columns) so the weight DMA is a plain contiguous copy.
  - wo_h     = W_O slice per head        [64, 1024]
  - Q^T/K^T computed as [head-pair 128, S]; V as [p, 65*4] with a ones
    column folded per head so the attention-value matmul also produces
    the softmax denominator row.
  - scores tile = K^T.T @ Q^T -> [p=128, q=512] in PSUM; causality is
    handled by skipping fully-masked 128-col blocks in scores/exp/AV and
    applying a multiplicative tril [128,128] to the probabilities of the
    true-diagonal blocks after exp (keeps DVE off the ACT feed path).
  - z^T accumulated in PSUM [65, 512] per head (row 64 = denominator l).
  - out[q,1024] = sum_hp znp_hp.T @ wo_hp as K=128 matmuls accumulated
    in PSUM over the two head pairs (matmul cost is N cycles regardless
    of K, so K=128 halves the O-proj time vs per-head K=64 matmuls).

Matmul compute dtype: bfloat16 (full-rate on TRN2; rel err ~6e-3 vs the
fp32 reference), fp32 accumulation in PSUM.

Schedule notes (why the structure looks the way it does):
  - ~20 dummy matmuls on scratch data at t=0 hold the PE busy through
    the HAM activity window, so the first real projection matmul runs
    at 2.4 GHz instead of the cold 1.2 GHz default.
  - The attention phase is paced by ACT's exp stream and the PE's
    score/AV matmuls; everything else hides under them. Pass A (pair 0)
    has ~25% PE slack at the exp pace; pair 1's projections (Q/K/V
    matmuls) are paced into exactly that slack. Pass B (pair 1) carries
    the per-chunk normalize + O-projection chains of earlier chunks in
    its slack.
  - Pass A processes q-chunks 3,2,1,0 (big first = most early slack for
    the deferred projections). Pass B runs 1,0,2,3: as soon as chunk
    qc's z/l finish (both pairs now known), its 1/l reciprocal is
    emitted on DVE, and its normalize (K=33 f32r selector matmuls
    broadcasting 1/l from rall + DVE/Pool muls) plus O-projection +
    output DMA are emitted as paced units into the following chunks'
    PE slack. Only the last chunk's (qc3) chain is exposed after the
    attention, and it runs on a warm PE: ACT-table reciprocal straight
    from the z PSUM denominator rows (pair 1) and from lall (pair 0),
    then selector broadcast, muls, and a fine-grained O-projection
    whose [128,1024] output DMAs chase the drains.
  - No stride-0 DMA broadcasts anywhere (their ~4us sequencer issue
    cost serialized the old tail); 1/l broadcast is always done with
    the selector matmul on the PE.
  - O-proj drains alternate ACT/DVE only in the post-attention tail
    (ACT is the exp pacer mid-pass, so in-pass chains drain DVE-only).
  - DMA head: issue order wqa, x0a, x0b, wqb, x1-x3, wk, x4-x7, wv,
    mask, sel, wo; wq/x0 are split so the first projection matmul
    waits on ~0.5MB instead of ~1.3MB behind the fixed ~10us DMA
    pipe-up. QT/KT run di-outer so the PE consumes each x chunk for
    ~1.7us, matching the ~1.4us/chunk DMA feed rate.
"""

import sys

if "/opt/trn_rl_repo" not in sys.path:
    sys.path.insert(0, "/opt/trn_rl_repo")

import numpy as np

import concourse.bass as bass
import concourse.mybir as mybir
import concourse.tile as tile
from concourse.bass_utils import run_bass_kernel_spmd

B = 2
S = 2048
D = 1024
NH = 16
DH = 64
N_CORES = 8
HPC = 4          # heads per core
HL = HPC * DH    # 256 local head dims
QC = 512         # q-chunk width
NQC = S // QC
NEG = -30000.0   # additive mask value; exp(NEG/8) == 0 in f32

F32 = mybir.dt.float32
F32R = mybir.dt.float32r
BF16 = mybir.dt.bfloat16
EXP = mybir.ActivationFunctionType.Exp
RCP = mybir.ActivationFunctionType.Reciprocal


def _act_rcp(nc, out, in_):
    """ACT-engine reciprocal via direct InstActivation emission. The bass
    wrapper refuses Reciprocal for accuracy reasons; here it only scales
    softmax denominators (l in [1, ~1e3]) where table accuracy is plenty
    for the 2e-2 tolerance, and it keeps the post-attention critical path
    off DVE's slow (3.3us) exact reciprocal."""
    eng = nc.scalar
    inputs = [eng.lower_ap(in_)]
    for arg in (0.0, 1.0, 0.0):  # bias, scale, alpha
        inputs.append(mybir.ImmediateValue(dtype=mybir.dt.float32, value=arg))
    return eng.add_instruction(
        mybir.InstActivation(
            name=eng.bass.get_next_instruction_name(),
            func=RCP,
            ins=inputs,
            outs=[eng.lower_ap(out)],
        )
    )


def _split_multiwait(nc, max_waits=1):
    """Walrus (CoreV3) rejects instructions carrying more than one sync
    wait; split extras into single-wait nops inserted before, same engine."""
    for f in nc.m.functions:
        for blk in f.blocks:
            insts = blk.instructions
            idx = 0
            while idx < len(insts):
                inst = insts[idx]
                si = getattr(inst, "sync_info", None)
                waits = list(si.on_wait) if si is not None else []
                if len(waits) > max_waits:
                    extra, keep = waits[:-max_waits], waits[-max_waits:]
                    si.on_wait = keep
                    for j, w in enumerate(extra):
                        nop = mybir.InstNoOp(
                            name=f"{inst.name}_sw{j}",
                            engine=inst.engine,
                            sync_info=mybir.SyncInfo(on_wait=[w], on_update=[]),
                            bass_nofuse=True,
                        )
                        insts.insert(idx, nop)
                        idx += 1
                idx += 1


def build_nc(stage=3):
    """stage 1: projections only (QT dumped to out); 2: + attention loop
    (zn dumped); 3: full kernel."""
    nc = bass.Bass("TRN2", target_bir_lowering=False, debug=False, num_devices=N_CORES)

    xT_d = nc.declare_dram_parameter("xT", [D, S], BF16, isOutput=False)
    wq_d = nc.declare_dram_parameter("wq", [128, 8 * HL], BF16, isOutput=False)
    wk_d = nc.declare_dram_parameter("wk", [128, 8 * HL], BF16, isOutput=False)
    wv_d = nc.declare_dram_parameter("wv", [128, 8 * HL], BF16, isOutput=False)
    wo_d = nc.declare_dram_parameter("wo", [HL, D], BF16, isOutput=False)
    mask_d = nc.declare_dram_parameter("mask", [128, 128], BF16, isOutput=False)
    sel_d = nc.declare_dram_parameter("sel", [128, 128], BF16, isOutput=False)
    out_d = nc.declare_dram_parameter("out", [S, D], BF16, isOutput=True)

    with tile.TileContext(nc) as tc:
        # ---- Phase 0: PE warm-up ----
        # ~20 garbage matmuls keep the PE busy through the HAM activity
        # window (free-running 4096-cycle @1.2GHz) while the input DMAs
        # stream; by the time wq/x0 land, the clock gate is at 8/8 so the
        # projections run at 2.4 GHz instead of the cold-default 1.2.
        with (
            tc.tile_pool(name="warm_sb", bufs=1) as warm_sb,
            tc.tile_pool(name="warm_ps", bufs=2, space="PSUM") as warm_ps,
        ):
            wsc = warm_sb.tile([128, 512], BF16, tag="wsc", name="wsc")
            nc.vector.memset(wsc[:, :], 0.0)
            wp = [warm_ps.tile([128, 512], F32, tag="wp", name="wp")
                  for _ in range(2)]
            for i in range(6):
                nc.tensor.matmul(
                    wp[i % 2][:, :], wsc[:, 0:128], wsc[:, :],
                    start=True, stop=True,
                )

        with (
            tc.tile_pool(name="live_sb", bufs=1) as live_sb,
            tc.tile_pool(name="att_sb", bufs=1) as att_sb,
        ):
            # Tensors that live through the whole kernel.
            QT = [live_sb.tile([128, S], BF16, tag=f"QT{hc}", name=f"QT{hc}") for hc in range(2)]
            KT = [live_sb.tile([128, S], BF16, tag=f"KT{hc}", name=f"KT{hc}") for hc in range(2)]
            # V with a ones column per head: 16 p-chunks x [V0|1|V1|1|V2|1|V3|1]
            V_sb = live_sb.tile([128, 16 * (HPC * 65)], BF16, tag="V", name="V")
            wop = [live_sb.tile([128, D], BF16, tag=f"wop{hp}", name=f"wop{hp}") for hp in range(2)]
            mask_t = live_sb.tile([128, 128], BF16, tag="mask", name="mask")

            ones_f = live_sb.tile([128, 64], F32, tag="ones_f", name="ones_f")
            nc.vector.memset(ones_f[:, :], 1.0)
            # K=2 broadcast selector (host constant, f32r): within a row
            # pair, even row -> output partitions 0-63, odd row -> 64-127.
            sel_t = live_sb.tile([128, 128], BF16, tag="sel", name="sel")

            # ---- Phase 1: DMAs + pass-A Q/K projections ----
            # xw_sb is scoped with the attention region: pair-1 projections
            # and both V projections read x/w tiles from inside the
            # attention passes.
            with tc.tile_pool(name="xw_sb", bufs=1) as xw_sb:
                # DMA issue order matters: the projection's first matmul
                # needs only wq + xT chunk 0; wk is needed ~14us in, wv
                # later, mask/wo much later. Interleave so the critical
                # pieces have the least transfer backlog in front of them.
                w_sb = {}
                w_tiles = {}
                for name in ("wk", "wv"):
                    w_tiles[name] = xw_sb.tile(
                        [128, 8 * HL], BF16, tag=f"{name}b", name=f"{name}b"
                    )

                def _w_dma(name, dram):
                    t = w_tiles[name]
                    nc.sync.dma_start(out=t[:, :], in_=dram[:, :])
                    w_sb[name] = t

                # wq and x0 are split so the first projection matmul waits
                # on ~0.5MB instead of ~1.3MB (the ~10us DMA pipe-up at
                # kernel start is fixed; the backlog in front of the first
                # dependencies is what's controllable).
                wqa = xw_sb.tile([128, 2 * HL], BF16, tag="wqa", name="wqa")
                wqb = xw_sb.tile([128, 6 * HL], BF16, tag="wqb", name="wqb")
                x0a = xw_sb.tile([128, 512], BF16, tag="x0a", name="x0a")
                x0b = xw_sb.tile([128, S - 512], BF16, tag="x0b", name="x0b")
                xT_t = [None] + [
                    xw_sb.tile([128, S], BF16, tag=f"x{di}", name=f"x{di}")
                    for di in range(1, 8)
                ]

                def _x_dma(di):
                    nc.sync.dma_start(
                        out=xT_t[di][:, :], in_=xT_d[di * 128:(di + 1) * 128, :]
                    )

                nc.sync.dma_start(out=wqa[:, :], in_=wq_d[:, 0:2 * HL])
                nc.sync.dma_start(out=x0a[:, :], in_=xT_d[0:128, 0:512])
                nc.sync.dma_start(out=x0b[:, :], in_=xT_d[0:128, 512:S])
                nc.sync.dma_start(out=wqb[:, :], in_=wq_d[:, 2 * HL:8 * HL])
                for di in range(1, 4):
                    _x_dma(di)
                _w_dma("wk", wk_d)
                for di in range(4, 8):
                    _x_dma(di)
                _w_dma("wv", wv_d)
                nc.sync.dma_start(out=mask_t[:, :], in_=mask_d[:, :])
                nc.sync.dma_start(out=sel_t[:, :], in_=sel_d[:, :])
                for hp in range(2):
                    nc.sync.dma_start(
                        out=wop[hp][:, :], in_=wo_d[hp * 128:(hp + 1) * 128, :]
                    )

                def w_t_slice(name, di, lo, hi):
                    if name == "wq":
                        if di < 2:
                            return wqa[:, di * HL + lo:di * HL + hi]
                        return wqb[:, (di - 2) * HL + lo:(di - 2) * HL + hi]
                    return w_sb[name][:, di * HL + lo:di * HL + hi]

                def x_slice(di, c0, c1):
                    if di == 0:
                        if c1 <= 512:
                            return x0a[:, c0:c1]
                        return x0b[:, c0 - 512:c1 - 512]
                    return xT_t[di][:, c0:c1]

                def v_drains(ps, pc, heads):
                    base = pc * (HPC * 65)
                    for i, h in enumerate(heads):
                        nc.vector.tensor_copy(
                            V_sb[:, base + h * 65: base + h * 65 + 64],
                            ps[:, i * 64:(i + 1) * 64],
                        )
                        nc.gpsimd.tensor_copy(
                            V_sb[:, base + h * 65 + 64: base + h * 65 + 65],
                            ones_f[:, 0:1],
                        )

                # Pass-A Q^T/K^T only, QT and KT interleaved per di chunk so
                # the PE consumes each x chunk for ~1.7us — matching the
                # ~1.4us/chunk DMA feed instead of outrunning it. V (both
                # pairs) streams into the attention passes below.
                with tc.tile_pool(name="proj_ps", bufs=8, space="PSUM") as proj_ps:
                    pss = {
                        (w, qt): proj_ps.tile([128, 512], F32, tag="pp", name="pp")
                        for w in range(2) for qt in range(4)
                    }
                    for di in range(8):
                        for w, wname in ((0, "wq"), (1, "wk")):
                            for qt in range(4):
                                nc.tensor.matmul(
                                    pss[(w, qt)][:, :],
                                    w_t_slice(wname, di, 0, 128),
                                    x_slice(di, qt * 512, (qt + 1) * 512),
                                    start=(di == 0),
                                    stop=(di == 7),
                                )
                    for w, dst in ((0, QT), (1, KT)):
                        for qt in range(4):
                            nc.vector.tensor_copy(
                                dst[0][:, qt * 512:(qt + 1) * 512],
                                pss[(w, qt)][:, :],
                            )
                    # V for BOTH pairs per (pc, di): one N=256 matmul reuses
                    # each x-chunk LDWEIGHTS across all four heads, halving
                    # the V-projection PE time vs per-pair N=128 matmuls.
                    for pc in range(16):
                        ps = proj_ps.tile([128, 512], F32, tag="pp", name="pp")
                        for di in range(8):
                            nc.tensor.matmul(
                                ps[:, 0:256],
                                x_slice(di, pc * 128, (pc + 1) * 128),
                                w_t_slice("wv", di, 0, 256),
                                start=(di == 0),
                                stop=(di == 7),
                            )
                        v_drains(ps, pc, (0, 1, 2, 3))

            # ---- Phase 2: two attention passes, one head pair each ----
            # zu: unnormalized z^T per head [64, S]; lall/rall: denominators
            # and their reciprocals, head h parked at partition 32h (pair 0
            # in partitions 0-63, pair 1 in 64-127 — passes never clash).
            # Pass A (pair 0) has PE slack at the exp pace; pair 1's
            # projections (Q/K/V matmuls) are paced into exactly that
            # slack. Pass B (pair 1) carries the normalize + O-projection
            # chains. PSUM: z 2 + scores 4 + pb 2 banks = 8.
            zu = [att_sb.tile([64, S], BF16, tag=f"zu{h}", name=f"zu{h}")
                  for h in range(HPC)]
            lall = att_sb.tile([128, S], F32, tag="lall", name="lall")
            # rall holds 1/l in bf16: the normalize K=33 selector matmuls
            # read it directly on the full-rate bf16 path.
            rall = att_sb.tile([128, S], BF16, tag="rall", name="rall")
            nc.vector.memset(lall[:, :], 1.0)
            znps = {}
            att_order_a = [3, 2, 1, 0]
            att_order_b = [1, 0, 2, 3]
            with (
                tc.tile_pool(name="z_ps", bufs=2, space="PSUM") as z_ps,
                tc.tile_pool(name="sc_ps", bufs=2, space="PSUM") as sc_ps,
                tc.tile_pool(name="pb_ps", bufs=2, space="PSUM") as pb_ps,
            ):
                # Deferred projection work, chopped into per-matmul
                # closures paced into each pass's PE slack. Pass A carries
                # its own V (front of the list — its AVs consume V chunk pt
                # at step pt+3, and the pacing completes chunk k by ~step
                # 0.9k) followed by pair 1's Q/K; pass B carries its own V.
                def v_units(units, lo, heads):
                    for pc in range(16):
                        box = {}
                        for di in range(8):
                            def mmv(di=di, pc=pc, box=box, lo=lo):
                                if di == 0:
                                    box["ps"] = pb_ps.tile(
                                        [128, 512], F32, tag="pb", name="pb"
                                    )
                                nc.tensor.matmul(
                                    box["ps"][:, 0:128],
                                    x_slice(di, pc * 128, (pc + 1) * 128),
                                    w_t_slice("wv", di, lo, lo + 128),
                                    start=(di == 0),
                                    stop=(di == 7),
                                )
                            units.append((128, mmv))

                        def drainv(pc=pc, box=box, heads=heads):
                            v_drains(box["ps"], pc, heads)
                        units.append((0, drainv))

                units_a = []
                for wname, dst in (("wq", QT), ("wk", KT)):
                    for qt in range(4):
                        box = {}
                        for di in range(8):
                            def mm(di=di, wname=wname, qt=qt, box=box):
                                if di == 0:
                                    box["ps"] = pb_ps.tile(
                                        [128, 512], F32, tag="pb", name="pb"
                                    )
                                nc.tensor.matmul(
                                    box["ps"][:, :],
                                    w_t_slice(wname, di, 128, 256),
                                    x_slice(di, qt * 512, (qt + 1) * 512),
                                    start=(di == 0),
                                    stop=(di == 7),
                                )
                            units_a.append((512, mm))

                        def drain(qt=qt, box=box, dst=dst):
                            nc.vector.tensor_copy(
                                dst[1][:, qt * 512:(qt + 1) * 512],
                                box["ps"][:, :],
                            )
                        units_a.append((0, drain))
                # V for pair 1 rides at the tail of pass A (no race: pass
                # B's AVs start a whole pass later), keeping pass B free
                # for the normalize/O-proj chains.
                v_units(units_a, 128, (2, 3))

                pb_budget = [0.0]

                def pb_emit(units, cycles):
                    pb_budget[0] += cycles
                    while units and pb_budget[0] > 0:
                        cyc, fn = units.pop(0)
                        fn()
                        pb_budget[0] -= cyc

                # Per-chunk normalize + O-projection chain for a pass-B
                # chunk (emitted as paced units into later chunks' PE
                # slack). The chain opens with its own 1/l, split into
                # four [128,128] DVE reciprocals (~0.9us each) so no
                # single op clogs the DVE FIFO in front of the mask muls
                # that feed the AV matmuls; output is bf16 straight into
                # rall (a quantization the bf16 staging copies already
                # applied in earlier revisions).
                def chain_units(qc, units):
                    q0 = qc * QC
                    nbox = {}
                    for j in range(4):
                        def rcp(j=j, q0=q0):
                            c = q0 + j * 128
                            with nc.allow_low_precision(reason="bf16 1/l"):
                                nc.vector.reciprocal(
                                    rall[:, c:c + 128], lall[:, c:c + 128]
                                )
                        units.append((400, rcp))
                    for hp in range(2):
                        def k33(hp=hp, nbox=nbox, q0=q0):
                            ps = pb_ps.tile([128, 512], F32, tag="pb", name="pb")
                            nc.tensor.matmul(
                                ps[:, :], sel_t[64 * hp:64 * hp + 33, :],
                                rall[64 * hp:64 * hp + 33, q0:q0 + QC],
                                start=True, stop=True,
                            )
                            nbox[("ps", hp)] = ps
                        units.append((512, k33))
                        for i in range(2):
                            def dr3(hp=hp, i=i, nbox=nbox):
                                rb = att_sb.tile([64, 512], BF16, tag="rbl",
                                                 name="rbl", bufs=8)
                                nc.vector.tensor_copy(
                                    rb[:, :],
                                    nbox[("ps", hp)][i * 64:(i + 1) * 64, :],
                                )
                                nbox[("rb", 2 * hp + i)] = rb
                            units.append((250, dr3))
                    for h in range(HPC):
                        def mul3(h=h, nbox=nbox, qc=qc, q0=q0):
                            hp, off = h // 2, (h % 2) * 64
                            eng = nc.vector if h % 2 == 0 else nc.gpsimd
                            eng.tensor_mul(
                                znps[qc][hp][off:off + 64, :],
                                zu[h][:, q0:q0 + QC], nbox[("rb", h)][:, :],
                            )
                        units.append((250, mul3))

                    for qg in range(2):
                        obox = {}
                        def alloc_ot(obox=obox):
                            obox["ot"] = att_sb.tile([128, 2048], BF16, tag="ot",
                                                     name="ot", bufs=4)
                        units.append((0, alloc_ot))
                        for t in range(2):
                            for dm in range(2):
                                box = {}
                                for hp in range(2):
                                    def omm(qg=qg, t=t, dm=dm, hp=hp, box=box, qc=qc):
                                        if hp == 0:
                                            box["ps"] = pb_ps.tile(
                                                [128, 512], F32, tag="pb", name="pb"
                                            )
                                        nc.tensor.matmul(
                                            box["ps"][:, :],
                                            znps[qc][hp][:, (qg * 2 + t) * 128:
                                                         (qg * 2 + t + 1) * 128],
                                            wop[hp][:, dm * 512:(dm + 1) * 512],
                                            start=(hp == 0),
                                            stop=(hp == 1),
                                        )
                                    units.append((512 if hp else 0, omm))

                                def odr(t=t, dm=dm, box=box, obox=obox):
                                    nc.vector.tensor_copy(
                                        obox["ot"][:, t * 1024 + dm * 512:
                                                   t * 1024 + (dm + 1) * 512],
                                        box["ps"][:, :],
                                    )
                                units.append((250, odr))

                        def odma(qg=qg, obox=obox, q0=q0):
                            r0 = q0 + qg * 256
                            nc.sync.dma_start(
                                out=out_d[r0:r0 + 256, :].rearrange(
                                    "(t p) d -> p t d", p=128
                                ),
                                in_=obox["ot"][:, :].rearrange(
                                    "p (t d) -> p t d", t=2
                                ),
                            )
                        units.append((0, odma))

                def attention_pass(php, order, gated=()):
                    # gated: list of (units, min_pos, min_step) — units
                    # only start once the pass reaches chunk-position
                    # min_pos AND step min_step within it (the in-order PE
                    # stalls on a premature sem wait, so every unit must
                    # have its inputs long ready when emitted).
                    heads = (2 * php, 2 * php + 1)
                    for pos, qc in enumerate(order if stage >= 2 else []):
                        q0 = qc * QC
                        npt = q0 // 128 + 4
                        last_qc = (pos == len(order) - 1)
                        zt = {h: z_ps.tile([65, 512], F32, tag="z", name="z")
                              for h in heads}
                        if php == 1:
                            # allocated before the step loop: the final
                            # chunk's pair-0 normalize units (gated into
                            # this chunk's slack) write znps[qc] mid-pass
                            znps[qc] = [
                                att_sb.tile([128, QC], BF16, tag=f"znp{hp}",
                                            name=f"znp{hp}", bufs=4)
                                for hp in range(2)
                            ]
                        Ps = {}

                        def emit_scores(pt):
                            p0 = pt * 128
                            jj = pt - q0 // 128  # >=0 means diagonal region
                            c0 = max(0, jj) * 128
                            scp = sc_ps.tile([128, 1024], F32, tag="sc", name="sc")
                            for i in range(2):
                                ho = i * 64
                                nc.tensor.matmul(
                                    scp[:, i * 512 + c0:(i + 1) * 512],
                                    KT[php][ho:ho + 64, p0:p0 + 128],
                                    QT[php][ho:ho + 64, q0 + c0:q0 + QC],
                                    start=True,
                                    stop=True,
                                    tile_position=(ho, 0),
                                )
                            Pp = att_sb.tile([128, 1024], BF16, tag="P",
                                             name="P", bufs=6)
                            nc.scalar.activation(Pp[:, c0:], scp[:, c0:], EXP,
                                                 scale=0.125)
                            if jj >= 0:
                                for i in range(2):
                                    blk = slice(i * 512 + jj * 128,
                                                i * 512 + (jj + 1) * 128)
                                    nc.vector.tensor_mul(
                                        Pp[:, blk], Pp[:, blk], mask_t[:, :]
                                    )
                            Ps[pt] = Pp

                        def emit_av(apt):
                            ac0 = max(0, apt - q0 // 128) * 128
                            Pp = Ps.pop(apt)
                            for i, h in enumerate(heads):
                                nc.tensor.matmul(
                                    zt[h][:, ac0:],
                                    V_sb[:, apt * (HPC * 65) + h * 65:
                                         apt * (HPC * 65) + (h + 1) * 65],
                                    Pp[:, i * 512 + ac0:(i + 1) * 512],
                                    start=(apt == 0),
                                    stop=(apt == npt - 1),
                                )

                        LAG = 3
                        for n in range(npt + LAG):
                            if n < npt:
                                emit_scores(n)
                                for units, min_pos, min_step in gated:
                                    # deferred work fills the PE slack
                                    # between this step's scores and the
                                    # lagged AV matmuls
                                    if units and (
                                        pos > min_pos
                                        or (pos == min_pos and n >= min_step)
                                    ):
                                        pb_emit(units, 1300)
                                        break
                            if n >= LAG:
                                emit_av(n - LAG)

                        if php == 0:
                            # Pass A: drain only; the reciprocal runs in
                            # pass B once both pairs' l are known.
                            for h in heads:
                                nc.vector.tensor_copy(
                                    lall[32 * h:32 * h + 1, q0:q0 + QC],
                                    zt[h][64:65, :],
                                )
                                nc.vector.tensor_copy(
                                    zu[h][:, q0:q0 + QC], zt[h][0:64, :]
                                )
                            continue

                        # ---- Pass B per-chunk l drains ----
                        # (each chunk's 1/l runs inside its gated chain;
                        # the final chunk's pair-1 1/l runs in phase 3)
                        for h in heads:
                            nc.vector.tensor_copy(
                                lall[32 * h:32 * h + 1, q0:q0 + QC],
                                zt[h][64:65, :],
                            )
                            nc.vector.tensor_copy(
                                zu[h][:, q0:q0 + QC], zt[h][0:64, :]
                            )

                attention_pass(0, att_order_a, gated=[(units_a, 0, 0)])
                pb_emit(units_a, 10**9)  # flush leftovers before pass B
                pb_budget[0] = 0.0

                # Pass B runs chunks 1,0,2,3; chunk qc's chain (normalize +
                # O-proj + out DMA) is paced into the chunks >= 2 positions
                # later (so every emitted unit's inputs are long ready and
                # never stall the in-order engine queues). The closures
                # dereference znps[qc]/rall lazily, so building the unit
                # lists up front is safe.
                units_c10 = []
                units_c2 = []
                units_f3 = []
                chain_units(att_order_b[0], units_c10)
                chain_units(att_order_b[1], units_c10)
                chain_units(att_order_b[2], units_c2)

                # Final chunk (qc3): its pair-0 denominators have been in
                # lall since pass A, so the whole pair-0 half of its
                # normalize (1/l, selector broadcast, muls into znp) runs
                # as gated units inside its own attention steps; only the
                # pair-1 half is exposed after the last AV (phase 3).
                oqf = att_order_b[-1]
                oq0 = oqf * QC
                fbox = {}

                for j in range(4):
                    def f3_rcp(j=j):
                        c = oq0 + j * 128
                        with nc.allow_low_precision(reason="bf16 1/l"):
                            nc.vector.reciprocal(
                                rall[0:64, c:c + 128], lall[0:64, c:c + 128]
                            )
                    units_f3.append((400, f3_rcp))

                def f3_k33(fbox=fbox):
                    ps = pb_ps.tile([128, 512], F32, tag="pb", name="pb")
                    nc.tensor.matmul(
                        ps[:, :], sel_t[0:33, :],
                        rall[0:33, oq0:oq0 + QC],
                        start=True, stop=True,
                    )
                    fbox["ps"] = ps
                units_f3.append((512, f3_k33))
                for i in range(2):
                    def f3_dr(i=i, fbox=fbox):
                        rb = att_sb.tile([64, 512], BF16, tag="rbl",
                                         name="rbl", bufs=8)
                        nc.vector.tensor_copy(
                            rb[:, :], fbox["ps"][i * 64:(i + 1) * 64, :]
                        )
                        fbox[("rb", i)] = rb
                    units_f3.append((250, f3_dr))
                for h in range(2):
                    def f3_mul(h=h, fbox=fbox):
                        eng = nc.vector if h % 2 == 0 else nc.gpsimd
                        eng.tensor_mul(
                            znps[oqf][0][h * 64:h * 64 + 64, :],
                            zu[h][:, oq0:oq0 + QC], fbox[("rb", h)][:, :],
                        )
                    units_f3.append((250, f3_mul))

                attention_pass(
                    1, att_order_b,
                    gated=[(units_c10, 2, 0), (units_f3, 3, 2),
                           (units_c2, 3, 6)],
                )

                pb_emit(units_c10, 10**9)
                pb_emit(units_f3, 10**9)
                pb_emit(units_c2, 10**9)

            # ---- Phase 3: final chunk's normalize + O-projection ----
            # (z/sc pools closed above so all 8 PSUM banks are free again)
            oqf = att_order_b[-1]
            with tc.tile_pool(name="o_ps", bufs=6, space="PSUM") as o_ps:
                def emit_oproj(qc, fine=False):
                    q0 = qc * QC
                    znp = znps[qc]
                    if fine:
                        # last output rows: 4 small DMAs chase the drains
                        for qs in range(4):
                            ot = att_sb.tile([128, 1024], BF16, tag="otf",
                                             name="otf", bufs=4)
                            for dm in range(2):
                                ps = o_ps.tile([128, 512], F32, tag="o", name="o")
                                for hp in range(2):
                                    nc.tensor.matmul(
                                        ps[:, :],
                                        znp[hp][:, qs * 128:(qs + 1) * 128],
                                        wop[hp][:, dm * 512:(dm + 1) * 512],
                                        start=(hp == 0),
                                        stop=(hp == 1),
                                    )
                                dst = ot[:, dm * 512:(dm + 1) * 512]
                                if dm == 0:
                                    nc.scalar.copy(dst, ps[:, :])
                                else:
                                    nc.vector.tensor_copy(dst, ps[:, :])
                            nc.sync.dma_start(
                                out=out_d[q0 + qs * 128: q0 + (qs + 1) * 128, :],
                                in_=ot[:, :],
                            )
                        return
                    for qg in range(2):  # 2 q-subchunks of 256 rows each
                        ot = att_sb.tile([128, 2048], BF16, tag="ot", name="ot", bufs=4)
                        for t in range(2):
                            qs = qg * 2 + t
                            for dm in range(2):
                                ps = o_ps.tile([128, 512], F32, tag="o", name="o")
                                for hp in range(2):
                                    nc.tensor.matmul(
                                        ps[:, :],
                                        znp[hp][:, qs * 128:(qs + 1) * 128],
                                        wop[hp][:, dm * 512:(dm + 1) * 512],
                                        start=(hp == 0),
                                        stop=(hp == 1),
                                    )
                                dst = ot[:, t * 1024 + dm * 512: t * 1024 + (dm + 1) * 512]
                                if dm == 0:
                                    nc.scalar.copy(dst, ps[:, :])
                                else:
                                    nc.vector.tensor_copy(dst, ps[:, :])
                        r0 = q0 + qg * 256
                        dst_ap = out_d[r0:r0 + 256, :].rearrange(
                            "(t p) d -> p t d", p=128
                        )
                        nc.sync.dma_start(
                            out=dst_ap,
                            in_=ot[:, :].rearrange("p (t d) -> p t d", t=2),
                        )

                def emit_latenorm_hp1(qc):
                    # Pair-1 half of the final chunk's normalize (pair 0
                    # already ran inside the chunk via units_f3): 1/l on
                    # ACT straight from lall rows 64-127 (the Reciprocal
                    # table load rides the ACT queue right after the last
                    # exp, hidden under the closing AV matmuls), K=33
                    # selector broadcast, ACT+DVE drains, DVE muls. ~4us
                    # serial after the last AV, on a warm PE that is
                    # still flushing chain(2)'s O-projection.
                    q0 = qc * QC
                    znp = znps[qc]
                    _act_rcp(
                        nc, rall[64:128, q0:q0 + QC],
                        lall[64:128, q0:q0 + QC],
                    )
                    rb_ps = o_ps.tile([128, 512], F32, tag="rbp",
                                      name="rb_ps", bufs=2)
                    nc.tensor.matmul(
                        rb_ps[:, :], sel_t[64:64 + 33, :],
                        rall[64:64 + 33, q0:q0 + QC],
                        start=True, stop=True,
                    )
                    rbs = []
                    for i in range(2):
                        rb = att_sb.tile([64, 512], BF16, tag="rbl",
                                         name="rbl", bufs=8)
                        if i == 0:
                            nc.scalar.copy(rb[:, :], rb_ps[0:64, :])
                        else:
                            nc.vector.tensor_copy(rb[:, :], rb_ps[64:128, :])
                        rbs.append(rb)
                    for h in (2, 3):
                        off = (h % 2) * 64
                        nc.vector.tensor_mul(
                            znp[1][off:off + 64, :],
                            zu[h][:, q0:q0 + QC], rbs[h - 2][:, :],
                        )

                if stage >= 3:
                    emit_latenorm_hp1(oqf)
                    emit_oproj(oqf, fine=True)

    _split_multiwait(nc)
    return nc


def _prep_in_maps(x, W_K, W_Q, W_V, W_O):
    x = np.asarray(x, dtype=np.float32)
    W_K = np.asarray(W_K, dtype=np.float32)
    W_Q = np.asarray(W_Q, dtype=np.float32)
    W_V = np.asarray(W_V, dtype=np.float32)
    W_O = np.asarray(W_O, dtype=np.float32)

    import ml_dtypes
    bf16 = ml_dtypes.bfloat16
    pp, qq = np.meshgrid(np.arange(128), np.arange(128), indexing="ij")
    mask = np.where(qq >= pp, 1.0, 0.0).astype(bf16)
    # K=33 broadcast selector: row p%64==0 -> out partitions 0-63,
    # row p%64==32 -> out partitions 64-127, all other rows ignored
    sel = np.where(((pp % 64 == 0) & (qq < 64)) | ((pp % 64 == 32) & (qq >= 64)),
                   1.0, 0.0).astype(bf16)

    def _pack_w(W, hs):
        # [heads, dh, D] -> [D, HL] (d rows, head-major cols) -> packed
        # [128, 8*HL] so the device DMA is a plain contiguous copy of the
        # on-SBUF layout [p, di, h].
        w = W[hs].transpose(2, 0, 1).reshape(D, HL)
        return np.ascontiguousarray(
            w.reshape(8, 128, HL).transpose(1, 0, 2).reshape(128, 8 * HL)
        ).astype(bf16)

    in_maps = []
    for c in range(N_CORES):
        b, g = c // 4, c % 4
        hs = slice(HPC * g, HPC * g + HPC)
        xT = np.ascontiguousarray(x[b].T).astype(bf16)
        wo = np.ascontiguousarray(W_O[:, HL * g:HL * g + HL].T).astype(bf16)
        in_maps.append(
            {
                "xT": xT,
                "wq": _pack_w(W_Q, hs),
                "wk": _pack_w(W_K, hs),
                "wv": _pack_w(W_V, hs),
                "wo": wo,
                "mask": mask,
                "sel": sel,
            }
        )
    return in_maps


_NC_CACHE = None


def _get_nc():
    global _NC_CACHE
    if _NC_CACHE is None:
        _NC_CACHE = build_nc()
    return _NC_CACHE


def _run(x, W_K, W_Q, W_V, W_O, trace=False):
    nc = _get_nc()
    in_maps = _prep_in_maps(x, W_K, W_Q, W_V, W_O)
    res = run_bass_kernel_spmd(
        nc, in_maps, core_ids=list(range(N_CORES)), trace=trace
    )
    partials = np.stack(
        [np.asarray(res.results[c]["out"]).astype(np.float32) for c in range(N_CORES)]
    )
    out = np.empty((B, S, D), dtype=np.float32)
    out[0] = partials[0:4].sum(axis=0)
    out[1] = partials[4:8].sum(axis=0)
    return out, res


def kernel(x, W_K, W_Q, W_V, W_O):
    out, _ = _run(x, W_K, W_Q, W_V, W_O, trace=False)
    return out


def run_traced(x, W_K, W_Q, W_V, W_O):
    """For test.py: returns (out, BassKernelResults with exec_time_ns)."""
    import types

    if "antenv.axon_hooks" not in sys.modules:
        try:
            from trn_agent_boot.trn_boot import _ntff_profile_via_ctypes

            hook = _ntff_profile_via_ctypes("/opt/axon/libaxon_pjrt.so")
            mod = types.ModuleType("antenv.axon_hooks")
            mod.get_axon_ntff_profile_hook = lambda: hook
            mod.set_axon_ntff_profile_hook = lambda h: None
            sys.modules["antenv.axon_hooks"] = mod
        except Exception:
            pass
    return _run(x, W_K, W_Q, W_V, W_O, trace=True)


# revision 23
# speedup vs baseline: 1.0771x; 1.0771x over previous
"""Distributed causal multi-head attention for 8 TRN2 NeuronCores.

Problem: x[2, 2048, 1024], 16 heads x 64 dim, causal softmax attention,
output projection. Sharding: tensor-parallel over (batch, head-group):
core c handles batch c//4 and heads [4*(c%4), 4*(c%4)+4). Each core
computes its 4 heads' attention plus the partial output projection
(sum over its heads); the host sums the 4 partials per batch.

On-device layout strategy (no transposes anywhere on device):
  - host feeds xT = x[b].T               [D=1024, S=2048]
  - wq/wk/wv = W[heads] host-packed as [128, 8*256] (partition-major,
    d-chunk-major columns) so the weight DMA is a plain contiguous copy.
  - wo_h     = W_O slice per head        [64, 1024]
  - Q^T/K^T computed as [head-pair 128, S]; V as [p, 65*4] with a ones
    column folded per head so the attention-value matmul also produces
    the softmax denominator row.
  - scores tile = K^T.T @ Q^T -> [p=128, q=512] in PSUM; causality is
    handled by skipping fully-masked 128-col blocks in scores/exp/AV and
    applying a multiplicative tril [128,128] to the probabilities of the
    true-diagonal blocks after exp (keeps DVE off the ACT feed path).
  - z^T accumulated in PSUM [65, 512] per head (row 64 = denominator l).
  - out[q,1024] = sum_hp znp_hp.T @ wo_hp as K=128 matmuls accumulated
    in PSUM over the two head pairs (matmul cost is N cycles regardless
    of K, so K=128 halves the O-proj time vs per-head K=64 matmuls).

Matmul compute dtype: bfloat16 (full-rate on TRN2; rel err ~6e-3 vs the
fp32 reference), fp32 accumulation in PSUM.

Schedule notes (why the structure looks the way it does):
  - ~20 dummy matmuls on scratch data at t=0 hold the PE busy through
    the HAM activity window, so the first real projection matmul runs
    at 2.4 GHz instead of the cold 1.2 GHz default.
  - The attention phase is paced by ACT's exp stream and the PE's
    score/AV matmuls; everything else hides under them. Pass A (pair 0)
    has ~25% PE slack at the exp pace; pair 1's projections (Q/K/V
    matmuls) are paced into exactly that slack. Pass B (pair 1) carries
    the per-chunk normalize + O-projection chains of earlier chunks in
    its slack.
  - Pass A processes q-chunks 3,2,1,0 (big first = most early slack for
    the deferred projections). Pass B runs 1,0,2,3: as soon as chunk
    qc's z/l finish (both pairs now known), its 1/l reciprocal is
    emitted on DVE, and its normalize (K=33 f32r selector matmuls
    broadcasting 1/l from rall + DVE/Pool muls) plus O-projection +
    output DMA are emitted as paced units into the following chunks'
    PE slack. Only the last chunk's (qc3) chain is exposed after the
    attention, and it runs on a warm PE: ACT-table reciprocal straight
    from the z PSUM denominator rows (pair 1) and from lall (pair 0),
    then selector broadcast, muls, and a fine-grained O-projection
    whose [128,1024] output DMAs chase the drains.
  - No stride-0 DMA broadcasts anywhere (their ~4us sequencer issue
    cost serialized the old tail); 1/l broadcast is always done with
    the selector matmul on the PE.
  - O-proj drains alternate ACT/DVE only in the post-attention tail
    (ACT is the exp pacer mid-pass, so in-pass chains drain DVE-only).
  - DMA head: issue order wqa, x0a, x0b, wqb, x1-x3, wk, x4-x7, wv,
    mask, sel, wo; wq/x0 are split so the first projection matmul
    waits on ~0.5MB instead of ~1.3MB behind the fixed ~10us DMA
    pipe-up. QT/KT run di-outer so the PE consumes each x chunk for
    ~1.7us, matching the ~1.4us/chunk DMA feed rate.
"""

import sys

if "/opt/trn_rl_repo" not in sys.path:
    sys.path.insert(0, "/opt/trn_rl_repo")

import numpy as np

import concourse.bass as bass
import concourse.mybir as mybir
import concourse.tile as tile
from concourse.bass_utils import run_bass_kernel_spmd

B = 2
S = 2048
D = 1024
NH = 16
DH = 64
N_CORES = 8
HPC = 4          # heads per core
HL = HPC * DH    # 256 local head dims
QC = 512         # q-chunk width
NQC = S // QC
NEG = -30000.0   # additive mask value; exp(NEG/8) == 0 in f32

F32 = mybir.dt.float32
F32R = mybir.dt.float32r
BF16 = mybir.dt.bfloat16
EXP = mybir.ActivationFunctionType.Exp
RCP = mybir.ActivationFunctionType.Reciprocal


def _act_rcp(nc, out, in_):
    """ACT-engine reciprocal via direct InstActivation emission. The bass
    wrapper refuses Reciprocal for accuracy reasons; here it only scales
    softmax denominators (l in [1, ~1e3]) where table accuracy is plenty
    for the 2e-2 tolerance, and it keeps the post-attention critical path
    off DVE's slow (3.3us) exact reciprocal."""
    eng = nc.scalar
    inputs = [eng.lower_ap(in_)]
    for arg in (0.0, 1.0, 0.0):  # bias, scale, alpha
        inputs.append(mybir.ImmediateValue(dtype=mybir.dt.float32, value=arg))
    return eng.add_instruction(
        mybir.InstActivation(
            name=eng.bass.get_next_instruction_name(),
            func=RCP,
            ins=inputs,
            outs=[eng.lower_ap(out)],
        )
    )


def _split_multiwait(nc, max_waits=1):
    """Walrus (CoreV3) rejects instructions carrying more than one sync
    wait; split extras into single-wait nops inserted before, same engine."""
    for f in nc.m.functions:
        for blk in f.blocks:
            insts = blk.instructions
            idx = 0
            while idx < len(insts):
                inst = insts[idx]
                si = getattr(inst, "sync_info", None)
                waits = list(si.on_wait) if si is not None else []
                if len(waits) > max_waits:
                    extra, keep = waits[:-max_waits], waits[-max_waits:]
                    si.on_wait = keep
                    for j, w in enumerate(extra):
                        nop = mybir.InstNoOp(
                            name=f"{inst.name}_sw{j}",
                            engine=inst.engine,
                            sync_info=mybir.SyncInfo(on_wait=[w], on_update=[]),
                            bass_nofuse=True,
                        )
                        insts.insert(idx, nop)
                        idx += 1
                idx += 1


def build_nc(stage=3):
    """stage 1: projections only (QT dumped to out); 2: + attention loop
    (zn dumped); 3: full kernel."""
    nc = bass.Bass("TRN2", target_bir_lowering=False, debug=False, num_devices=N_CORES)

    xT_d = nc.declare_dram_parameter("xT", [D, S], BF16, isOutput=False)
    wq_d = nc.declare_dram_parameter("wq", [128, 8 * HL], BF16, isOutput=False)
    wk_d = nc.declare_dram_parameter("wk", [128, 8 * HL], BF16, isOutput=False)
    wv_d = nc.declare_dram_parameter("wv", [128, 8 * HL], BF16, isOutput=False)
    wo_d = nc.declare_dram_parameter("wo", [HL, D], BF16, isOutput=False)
    mask_d = nc.declare_dram_parameter("mask", [128, 128], BF16, isOutput=False)
    sel_d = nc.declare_dram_parameter("sel", [128, 128], BF16, isOutput=False)
    out_d = nc.declare_dram_parameter("out", [S, D], BF16, isOutput=True)

    with tile.TileContext(nc) as tc:
        # ---- Phase 0: PE warm-up ----
        # ~20 garbage matmuls keep the PE busy through the HAM activity
        # window (free-running 4096-cycle @1.2GHz) while the input DMAs
        # stream; by the time wq/x0 land, the clock gate is at 8/8 so the
        # projections run at 2.4 GHz instead of the cold-default 1.2.
        with (
            tc.tile_pool(name="warm_sb", bufs=1) as warm_sb,
            tc.tile_pool(name="warm_ps", bufs=2, space="PSUM") as warm_ps,
        ):
            wsc = warm_sb.tile([128, 512], BF16, tag="wsc", name="wsc")
            nc.vector.memset(wsc[:, :], 0.0)
            wp = [warm_ps.tile([128, 512], F32, tag="wp", name="wp")
                  for _ in range(2)]
            for i in range(20):
                nc.tensor.matmul(
                    wp[i % 2][:, :], wsc[:, 0:128], wsc[:, :],
                    start=True, stop=True,
                )

        with (
            tc.tile_pool(name="live_sb", bufs=1) as live_sb,
            tc.tile_pool(name="att_sb", bufs=1) as att_sb,
        ):
            # Tensors that live through the whole kernel.
            QT = [live_sb.tile([128, S], BF16, tag=f"QT{hc}", name=f"QT{hc}") for hc in range(2)]
            KT = [live_sb.tile([128, S], BF16, tag=f"KT{hc}", name=f"KT{hc}") for hc in range(2)]
            # V with a ones column per head: 16 p-chunks x [V0|1|V1|1|V2|1|V3|1]
            V_sb = live_sb.tile([128, 16 * (HPC * 65)], BF16, tag="V", name="V")
            wop = [live_sb.tile([128, D], BF16, tag=f"wop{hp}", name=f"wop{hp}") for hp in range(2)]
            mask_t = live_sb.tile([128, 128], BF16, tag="mask", name="mask")

            ones_f = live_sb.tile([128, 64], F32, tag="ones_f", name="ones_f")
            nc.vector.memset(ones_f[:, :], 1.0)
            # K=2 broadcast selector (host constant, f32r): within a row
            # pair, even row -> output partitions 0-63, odd row -> 64-127.
            sel_t = live_sb.tile([128, 128], BF16, tag="sel", name="sel")

            # ---- Phase 1: DMAs + pass-A Q/K projections ----
            # xw_sb is scoped with the attention region: pair-1 projections
            # and both V projections read x/w tiles from inside the
            # attention passes.
            with tc.tile_pool(name="xw_sb", bufs=1) as xw_sb:
                # DMA issue order matters: the projection's first matmul
                # needs only wq + xT chunk 0; wk is needed ~14us in, wv
                # later, mask/wo much later. Interleave so the critical
                # pieces have the least transfer backlog in front of them.
                w_sb = {}
                w_tiles = {}
                for name in ("wk", "wv"):
                    w_tiles[name] = xw_sb.tile(
                        [128, 8 * HL], BF16, tag=f"{name}b", name=f"{name}b"
                    )

                def _w_dma(name, dram):
                    t = w_tiles[name]
                    nc.sync.dma_start(out=t[:, :], in_=dram[:, :])
                    w_sb[name] = t

                # wq and x0 are split so the first projection matmul waits
                # on ~0.5MB instead of ~1.3MB (the ~10us DMA pipe-up at
                # kernel start is fixed; the backlog in front of the first
                # dependencies is what's controllable).
                wqa = xw_sb.tile([128, 2 * HL], BF16, tag="wqa", name="wqa")
                wqb = xw_sb.tile([128, 6 * HL], BF16, tag="wqb", name="wqb")
                x0a = xw_sb.tile([128, 512], BF16, tag="x0a", name="x0a")
                x0b = xw_sb.tile([128, S - 512], BF16, tag="x0b", name="x0b")
                xT_t = [None] + [
                    xw_sb.tile([128, S], BF16, tag=f"x{di}", name=f"x{di}")
                    for di in range(1, 8)
                ]

                def _x_dma(di):
                    nc.sync.dma_start(
                        out=xT_t[di][:, :], in_=xT_d[di * 128:(di + 1) * 128, :]
                    )

                nc.sync.dma_start(out=wqa[:, :], in_=wq_d[:, 0:2 * HL])
                nc.sync.dma_start(out=x0a[:, :], in_=xT_d[0:128, 0:512])
                nc.sync.dma_start(out=x0b[:, :], in_=xT_d[0:128, 512:S])
                nc.sync.dma_start(out=wqb[:, :], in_=wq_d[:, 2 * HL:8 * HL])
                for di in range(1, 4):
                    _x_dma(di)
                _w_dma("wk", wk_d)
                for di in range(4, 8):
                    _x_dma(di)
                _w_dma("wv", wv_d)
                nc.sync.dma_start(out=mask_t[:, :], in_=mask_d[:, :])
                nc.sync.dma_start(out=sel_t[:, :], in_=sel_d[:, :])
                for hp in range(2):
                    nc.sync.dma_start(
                        out=wop[hp][:, :], in_=wo_d[hp * 128:(hp + 1) * 128, :]
                    )

                def w_t_slice(name, di, lo, hi):
                    if name == "wq":
                        if di < 2:
                            return wqa[:, di * HL + lo:di * HL + hi]
                        return wqb[:, (di - 2) * HL + lo:(di - 2) * HL + hi]
                    return w_sb[name][:, di * HL + lo:di * HL + hi]

                def x_slice(di, c0, c1):
                    if di == 0:
                        if c1 <= 512:
                            return x0a[:, c0:c1]
                        return x0b[:, c0 - 512:c1 - 512]
                    return xT_t[di][:, c0:c1]

                def v_drains(ps, pc, heads):
                    base = pc * (HPC * 65)
                    for i, h in enumerate(heads):
                        nc.vector.tensor_copy(
                            V_sb[:, base + h * 65: base + h * 65 + 64],
                            ps[:, i * 64:(i + 1) * 64],
                        )
                        nc.gpsimd.tensor_copy(
                            V_sb[:, base + h * 65 + 64: base + h * 65 + 65],
                            ones_f[:, 0:1],
                        )

                # Pass-A Q^T/K^T only, QT and KT interleaved per di chunk so
                # the PE consumes each x chunk for ~1.7us — matching the
                # ~1.4us/chunk DMA feed instead of outrunning it. V (both
                # pairs) streams into the attention passes below.
                with tc.tile_pool(name="proj_ps", bufs=8, space="PSUM") as proj_ps:
                    pss = {
                        (w, qt): proj_ps.tile([128, 512], F32, tag="pp", name="pp")
                        for w in range(2) for qt in range(4)
                    }
                    for di in range(8):
                        for w, wname in ((0, "wq"), (1, "wk")):
                            for qt in range(4):
                                nc.tensor.matmul(
                                    pss[(w, qt)][:, :],
                                    w_t_slice(wname, di, 0, 128),
                                    x_slice(di, qt * 512, (qt + 1) * 512),
                                    start=(di == 0),
                                    stop=(di == 7),
                                )
                    for w, dst in ((0, QT), (1, KT)):
                        for qt in range(4):
                            nc.vector.tensor_copy(
                                dst[0][:, qt * 512:(qt + 1) * 512],
                                pss[(w, qt)][:, :],
                            )
                    for pc in range(16):
                        ps = proj_ps.tile([128, 512], F32, tag="pp", name="pp")
                        for di in range(8):
                            nc.tensor.matmul(
                                ps[:, 0:128],
                                x_slice(di, pc * 128, (pc + 1) * 128),
                                w_t_slice("wv", di, 0, 128),
                                start=(di == 0),
                                stop=(di == 7),
                            )
                        v_drains(ps, pc, (0, 1))

            # ---- Phase 2: two attention passes, one head pair each ----
            # zu: unnormalized z^T per head [64, S]; lall/rall: denominators
            # and their reciprocals, head h parked at partition 32h (pair 0
            # in partitions 0-63, pair 1 in 64-127 — passes never clash).
            # Pass A (pair 0) has PE slack at the exp pace; pair 1's
            # projections (Q/K/V matmuls) are paced into exactly that
            # slack. Pass B (pair 1) carries the normalize + O-projection
            # chains. PSUM: z 2 + scores 4 + pb 2 banks = 8.
            zu = [att_sb.tile([64, S], BF16, tag=f"zu{h}", name=f"zu{h}")
                  for h in range(HPC)]
            lall = att_sb.tile([128, S], F32, tag="lall", name="lall")
            # rall holds 1/l in bf16: the normalize K=33 selector matmuls
            # read it directly on the full-rate bf16 path.
            rall = att_sb.tile([128, S], BF16, tag="rall", name="rall")
            nc.vector.memset(lall[:, :], 1.0)
            nc.vector.memset(rall[:, :], 1.0)
            znps = {}
            att_order_a = [3, 2, 1, 0]
            att_order_b = [1, 0, 2, 3]
            with (
                tc.tile_pool(name="z_ps", bufs=2, space="PSUM") as z_ps,
                tc.tile_pool(name="sc_ps", bufs=2, space="PSUM") as sc_ps,
                tc.tile_pool(name="pb_ps", bufs=2, space="PSUM") as pb_ps,
            ):
                # Deferred projection work, chopped into per-matmul
                # closures paced into each pass's PE slack. Pass A carries
                # its own V (front of the list — its AVs consume V chunk pt
                # at step pt+3, and the pacing completes chunk k by ~step
                # 0.9k) followed by pair 1's Q/K; pass B carries its own V.
                def v_units(units, lo, heads):
                    for pc in range(16):
                        box = {}
                        for di in range(8):
                            def mmv(di=di, pc=pc, box=box, lo=lo):
                                if di == 0:
                                    box["ps"] = pb_ps.tile(
                                        [128, 512], F32, tag="pb", name="pb"
                                    )
                                nc.tensor.matmul(
                                    box["ps"][:, 0:128],
                                    x_slice(di, pc * 128, (pc + 1) * 128),
                                    w_t_slice("wv", di, lo, lo + 128),
                                    start=(di == 0),
                                    stop=(di == 7),
                                )
                            units.append((128, mmv))

                        def drainv(pc=pc, box=box, heads=heads):
                            v_drains(box["ps"], pc, heads)
                        units.append((0, drainv))

                units_a = []
                for wname, dst in (("wq", QT), ("wk", KT)):
                    for qt in range(4):
                        box = {}
                        for di in range(8):
                            def mm(di=di, wname=wname, qt=qt, box=box):
                                if di == 0:
                                    box["ps"] = pb_ps.tile(
                                        [128, 512], F32, tag="pb", name="pb"
                                    )
                                nc.tensor.matmul(
                                    box["ps"][:, :],
                                    w_t_slice(wname, di, 128, 256),
                                    x_slice(di, qt * 512, (qt + 1) * 512),
                                    start=(di == 0),
                                    stop=(di == 7),
                                )
                            units_a.append((512, mm))

                        def drain(qt=qt, box=box, dst=dst):
                            nc.vector.tensor_copy(
                                dst[1][:, qt * 512:(qt + 1) * 512],
                                box["ps"][:, :],
                            )
                        units_a.append((0, drain))
                # V for pair 1 rides at the tail of pass A (no race: pass
                # B's AVs start a whole pass later), keeping pass B free
                # for the normalize/O-proj chains.
                v_units(units_a, 128, (2, 3))

                pb_budget = [0.0]

                def pb_emit(units, cycles):
                    pb_budget[0] += cycles
                    while units and pb_budget[0] > 0:
                        cyc, fn = units.pop(0)
                        fn()
                        pb_budget[0] -= cyc

                # Per-chunk normalize + O-projection chain for a pass-B
                # chunk (emitted as paced units into later chunks' PE
                # slack). The chain opens with its own 1/l, split into
                # four [128,128] DVE reciprocals (~0.9us each) so no
                # single op clogs the DVE FIFO in front of the mask muls
                # that feed the AV matmuls; output is bf16 straight into
                # rall (a quantization the bf16 staging copies already
                # applied in earlier revisions).
                def chain_units(qc, units):
                    q0 = qc * QC
                    nbox = {}
                    for j in range(4):
                        def rcp(j=j, q0=q0):
                            c = q0 + j * 128
                            with nc.allow_low_precision(reason="bf16 1/l"):
                                nc.vector.reciprocal(
                                    rall[:, c:c + 128], lall[:, c:c + 128]
                                )
                        units.append((400, rcp))
                    for hp in range(2):
                        def k33(hp=hp, nbox=nbox, q0=q0):
                            ps = pb_ps.tile([128, 512], F32, tag="pb", name="pb")
                            nc.tensor.matmul(
                                ps[:, :], sel_t[64 * hp:64 * hp + 33, :],
                                rall[64 * hp:64 * hp + 33, q0:q0 + QC],
                                start=True, stop=True,
                            )
                            nbox[("ps", hp)] = ps
                        units.append((512, k33))
                        for i in range(2):
                            def dr3(hp=hp, i=i, nbox=nbox):
                                rb = att_sb.tile([64, 512], BF16, tag="rbl",
                                                 name="rbl", bufs=8)
                                nc.vector.tensor_copy(
                                    rb[:, :],
                                    nbox[("ps", hp)][i * 64:(i + 1) * 64, :],
                                )
                                nbox[("rb", 2 * hp + i)] = rb
                            units.append((250, dr3))
                    for h in range(HPC):
                        def mul3(h=h, nbox=nbox, qc=qc, q0=q0):
                            hp, off = h // 2, (h % 2) * 64
                            eng = nc.vector if h % 2 == 0 else nc.gpsimd
                            eng.tensor_mul(
                                znps[qc][hp][off:off + 64, :],
                                zu[h][:, q0:q0 + QC], nbox[("rb", h)][:, :],
                            )
                        units.append((250, mul3))

                    for qg in range(2):
                        obox = {}
                        def alloc_ot(obox=obox):
                            obox["ot"] = att_sb.tile([128, 2048], BF16, tag="ot",
                                                     name="ot", bufs=4)
                        units.append((0, alloc_ot))
                        for t in range(2):
                            for dm in range(2):
                                box = {}
                                for hp in range(2):
                                    def omm(qg=qg, t=t, dm=dm, hp=hp, box=box, qc=qc):
                                        if hp == 0:
                                            box["ps"] = pb_ps.tile(
                                                [128, 512], F32, tag="pb", name="pb"
                                            )
                                        nc.tensor.matmul(
                                            box["ps"][:, :],
                                            znps[qc][hp][:, (qg * 2 + t) * 128:
                                                         (qg * 2 + t + 1) * 128],
                                            wop[hp][:, dm * 512:(dm + 1) * 512],
                                            start=(hp == 0),
                                            stop=(hp == 1),
                                        )
                                    units.append((512 if hp else 0, omm))

                                def odr(t=t, dm=dm, box=box, obox=obox):
                                    dst = obox["ot"][:, t * 1024 + dm * 512:
                                                     t * 1024 + (dm + 1) * 512]
                                    if dm == 0:
                                        nc.scalar.copy(dst, box["ps"][:, :])
                                    else:
                                        nc.vector.tensor_copy(dst, box["ps"][:, :])
                                units.append((250, odr))

                        def odma(qg=qg, obox=obox, q0=q0):
                            r0 = q0 + qg * 256
                            nc.sync.dma_start(
                                out=out_d[r0:r0 + 256, :].rearrange(
                                    "(t p) d -> p t d", p=128
                                ),
                                in_=obox["ot"][:, :].rearrange(
                                    "p (t d) -> p t d", t=2
                                ),
                            )
                        units.append((0, odma))

                def attention_pass(php, order, gated=()):
                    # gated: list of (units, min_pos, min_step) — units
                    # only start once the pass reaches chunk-position
                    # min_pos AND step min_step within it (the in-order PE
                    # stalls on a premature sem wait, so every unit must
                    # have its inputs long ready when emitted).
                    heads = (2 * php, 2 * php + 1)
                    for pos, qc in enumerate(order if stage >= 2 else []):
                        q0 = qc * QC
                        npt = q0 // 128 + 4
                        last_qc = (pos == len(order) - 1)
                        zt = {h: z_ps.tile([65, 512], F32, tag="z", name="z")
                              for h in heads}
                        if php == 1:
                            # allocated before the step loop: the final
                            # chunk's pair-0 normalize units (gated into
                            # this chunk's slack) write znps[qc] mid-pass
                            znps[qc] = [
                                att_sb.tile([128, QC], BF16, tag=f"znp{hp}",
                                            name=f"znp{hp}", bufs=4)
                                for hp in range(2)
                            ]
                        Ps = {}

                        def emit_scores(pt):
                            p0 = pt * 128
                            jj = pt - q0 // 128  # >=0 means diagonal region
                            c0 = max(0, jj) * 128
                            scp = sc_ps.tile([128, 1024], F32, tag="sc", name="sc")
                            for i in range(2):
                                ho = i * 64
                                nc.tensor.matmul(
                                    scp[:, i * 512 + c0:(i + 1) * 512],
                                    KT[php][ho:ho + 64, p0:p0 + 128],
                                    QT[php][ho:ho + 64, q0 + c0:q0 + QC],
                                    start=True,
                                    stop=True,
                                    tile_position=(ho, 0),
                                )
                            Pp = att_sb.tile([128, 1024], BF16, tag="P",
                                             name="P", bufs=6)
                            nc.scalar.activation(Pp[:, c0:], scp[:, c0:], EXP,
                                                 scale=0.125)
                            if jj >= 0:
                                for i in range(2):
                                    blk = slice(i * 512 + jj * 128,
                                                i * 512 + (jj + 1) * 128)
                                    nc.vector.tensor_mul(
                                        Pp[:, blk], Pp[:, blk], mask_t[:, :]
                                    )
                            Ps[pt] = Pp

                        def emit_av(apt):
                            ac0 = max(0, apt - q0 // 128) * 128
                            Pp = Ps.pop(apt)
                            for i, h in enumerate(heads):
                                nc.tensor.matmul(
                                    zt[h][:, ac0:],
                                    V_sb[:, apt * (HPC * 65) + h * 65:
                                         apt * (HPC * 65) + (h + 1) * 65],
                                    Pp[:, i * 512 + ac0:(i + 1) * 512],
                                    start=(apt == 0),
                                    stop=(apt == npt - 1),
                                )

                        LAG = 3
                        for n in range(npt + LAG):
                            if n < npt:
                                emit_scores(n)
                                for units, min_pos, min_step in gated:
                                    # deferred work fills the PE slack
                                    # between this step's scores and the
                                    # lagged AV matmuls
                                    if units and (
                                        pos > min_pos
                                        or (pos == min_pos and n >= min_step)
                                    ):
                                        pb_emit(units, 1300)
                                        break
                            if n >= LAG:
                                emit_av(n - LAG)

                        if php == 0:
                            # Pass A: drain only; the reciprocal runs in
                            # pass B once both pairs' l are known.
                            for h in heads:
                                nc.vector.tensor_copy(
                                    lall[32 * h:32 * h + 1, q0:q0 + QC],
                                    zt[h][64:65, :],
                                )
                                nc.vector.tensor_copy(
                                    zu[h][:, q0:q0 + QC], zt[h][0:64, :]
                                )
                            continue

                        # ---- Pass B per-chunk l drains ----
                        # (each chunk's 1/l runs inside its gated chain;
                        # the final chunk's pair-1 1/l runs right here on
                        # ACT, straight from the z PSUM denominator rows —
                        # the Reciprocal table load rides the ACT queue
                        # after the last exp, hidden under the closing AVs)
                        for h in heads:
                            if last_qc:
                                _act_rcp(
                                    nc, rall[32 * h:32 * h + 1, q0:q0 + QC],
                                    zt[h][64:65, :],
                                )
                            else:
                                nc.vector.tensor_copy(
                                    lall[32 * h:32 * h + 1, q0:q0 + QC],
                                    zt[h][64:65, :],
                                )
                            nc.vector.tensor_copy(
                                zu[h][:, q0:q0 + QC], zt[h][0:64, :]
                            )

                attention_pass(0, att_order_a, gated=[(units_a, 0, 0)])
                pb_emit(units_a, 10**9)  # flush leftovers before pass B
                pb_budget[0] = 0.0

                # Pass B runs chunks 1,0,2,3; chunk qc's chain (normalize +
                # O-proj + out DMA) is paced into the chunks >= 2 positions
                # later (so every emitted unit's inputs are long ready and
                # never stall the in-order engine queues). The closures
                # dereference znps[qc]/rall lazily, so building the unit
                # lists up front is safe.
                units_c10 = []
                units_c2 = []
                units_f3 = []
                chain_units(att_order_b[0], units_c10)
                chain_units(att_order_b[1], units_c10)
                chain_units(att_order_b[2], units_c2)

                # Final chunk (qc3): its pair-0 denominators have been in
                # lall since pass A, so the whole pair-0 half of its
                # normalize (1/l, selector broadcast, muls into znp) runs
                # as gated units inside its own attention steps; only the
                # pair-1 half is exposed after the last AV (phase 3).
                oqf = att_order_b[-1]
                oq0 = oqf * QC
                fbox = {}

                for j in range(4):
                    def f3_rcp(j=j):
                        c = oq0 + j * 128
                        with nc.allow_low_precision(reason="bf16 1/l"):
                            nc.vector.reciprocal(
                                rall[0:64, c:c + 128], lall[0:64, c:c + 128]
                            )
                    units_f3.append((400, f3_rcp))

                def f3_k33(fbox=fbox):
                    ps = pb_ps.tile([128, 512], F32, tag="pb", name="pb")
                    nc.tensor.matmul(
                        ps[:, :], sel_t[0:33, :],
                        rall[0:33, oq0:oq0 + QC],
                        start=True, stop=True,
                    )
                    fbox["ps"] = ps
                units_f3.append((512, f3_k33))
                for i in range(2):
                    def f3_dr(i=i, fbox=fbox):
                        rb = att_sb.tile([64, 512], BF16, tag="rbl",
                                         name="rbl", bufs=8)
                        nc.vector.tensor_copy(
                            rb[:, :], fbox["ps"][i * 64:(i + 1) * 64, :]
                        )
                        fbox[("rb", i)] = rb
                    units_f3.append((250, f3_dr))
                for h in range(2):
                    def f3_mul(h=h, fbox=fbox):
                        eng = nc.vector if h % 2 == 0 else nc.gpsimd
                        eng.tensor_mul(
                            znps[oqf][0][h * 64:h * 64 + 64, :],
                            zu[h][:, oq0:oq0 + QC], fbox[("rb", h)][:, :],
                        )
                    units_f3.append((250, f3_mul))

                attention_pass(
                    1, att_order_b,
                    gated=[(units_c10, 2, 0), (units_f3, 3, 2),
                           (units_c2, 3, 9)],
                )

                pb_emit(units_c10, 10**9)
                pb_emit(units_f3, 10**9)
                pb_emit(units_c2, 10**9)

            # ---- Phase 3: final chunk's normalize + O-projection ----
            # (z/sc pools closed above so all 8 PSUM banks are free again)
            oqf = att_order_b[-1]
            with tc.tile_pool(name="o_ps", bufs=6, space="PSUM") as o_ps:
                def emit_oproj(qc, fine=False):
                    q0 = qc * QC
                    znp = znps[qc]
                    if fine:
                        # last output rows: 4 small DMAs chase the drains
                        for qs in range(4):
                            ot = att_sb.tile([128, 1024], BF16, tag="otf",
                                             name="otf", bufs=4)
                            for dm in range(2):
                                ps = o_ps.tile([128, 512], F32, tag="o", name="o")
                                for hp in range(2):
                                    nc.tensor.matmul(
                                        ps[:, :],
                                        znp[hp][:, qs * 128:(qs + 1) * 128],
                                        wop[hp][:, dm * 512:(dm + 1) * 512],
                                        start=(hp == 0),
                                        stop=(hp == 1),
                                    )
                                dst = ot[:, dm * 512:(dm + 1) * 512]
                                if dm == 0:
                                    nc.scalar.copy(dst, ps[:, :])
                                else:
                                    nc.vector.tensor_copy(dst, ps[:, :])
                            nc.sync.dma_start(
                                out=out_d[q0 + qs * 128: q0 + (qs + 1) * 128, :],
                                in_=ot[:, :],
                            )
                        return
                    for qg in range(2):  # 2 q-subchunks of 256 rows each
                        ot = att_sb.tile([128, 2048], BF16, tag="ot", name="ot", bufs=4)
                        for t in range(2):
                            qs = qg * 2 + t
                            for dm in range(2):
                                ps = o_ps.tile([128, 512], F32, tag="o", name="o")
                                for hp in range(2):
                                    nc.tensor.matmul(
                                        ps[:, :],
                                        znp[hp][:, qs * 128:(qs + 1) * 128],
                                        wop[hp][:, dm * 512:(dm + 1) * 512],
                                        start=(hp == 0),
                                        stop=(hp == 1),
                                    )
                                dst = ot[:, t * 1024 + dm * 512: t * 1024 + (dm + 1) * 512]
                                if dm == 0:
                                    nc.scalar.copy(dst, ps[:, :])
                                else:
                                    nc.vector.tensor_copy(dst, ps[:, :])
                        r0 = q0 + qg * 256
                        dst_ap = out_d[r0:r0 + 256, :].rearrange(
                            "(t p) d -> p t d", p=128
                        )
                        nc.sync.dma_start(
                            out=dst_ap,
                            in_=ot[:, :].rearrange("p (t d) -> p t d", t=2),
                        )

                def emit_latenorm_hp1(qc):
                    # Pair-1 half of the final chunk's normalize (pair 0
                    # already ran inside the chunk via units_f3): 1/l on
                    # ACT straight from lall rows 64-127 (the Reciprocal
                    # table load rides the ACT queue right after the last
                    # exp, hidden under the closing AV matmuls), K=33
                    # selector broadcast, ACT+DVE drains, DVE muls. ~4us
                    # serial after the last AV, on a warm PE that is
                    # still flushing chain(2)'s O-projection.
                    q0 = qc * QC
                    znp = znps[qc]
                    rb_ps = o_ps.tile([128, 512], F32, tag="rbp",
                                      name="rb_ps", bufs=2)
                    nc.tensor.matmul(
                        rb_ps[:, :], sel_t[64:64 + 33, :],
                        rall[64:64 + 33, q0:q0 + QC],
                        start=True, stop=True,
                    )
                    rbs = []
                    for i in range(2):
                        rb = att_sb.tile([64, 512], BF16, tag="rbl",
                                         name="rbl", bufs=8)
                        if i == 0:
                            nc.scalar.copy(rb[:, :], rb_ps[0:64, :])
                        else:
                            nc.vector.tensor_copy(rb[:, :], rb_ps[64:128, :])
                        rbs.append(rb)
                    for h in (2, 3):
                        off = (h % 2) * 64
                        nc.vector.tensor_mul(
                            znp[1][off:off + 64, :],
                            zu[h][:, q0:q0 + QC], rbs[h - 2][:, :],
                        )

                if stage >= 3:
                    emit_latenorm_hp1(oqf)
                    emit_oproj(oqf, fine=True)

    _split_multiwait(nc)
    return nc


def _prep_in_maps(x, W_K, W_Q, W_V, W_O):
    x = np.asarray(x, dtype=np.float32)
    W_K = np.asarray(W_K, dtype=np.float32)
    W_Q = np.asarray(W_Q, dtype=np.float32)
    W_V = np.asarray(W_V, dtype=np.float32)
    W_O = np.asarray(W_O, dtype=np.float32)

    import ml_dtypes
    bf16 = ml_dtypes.bfloat16
    pp, qq = np.meshgrid(np.arange(128), np.arange(128), indexing="ij")
    mask = np.where(qq >= pp, 1.0, 0.0).astype(bf16)
    # K=33 broadcast selector: row p%64==0 -> out partitions 0-63,
    # row p%64==32 -> out partitions 64-127, all other rows ignored
    sel = np.where(((pp % 64 == 0) & (qq < 64)) | ((pp % 64 == 32) & (qq >= 64)),
                   1.0, 0.0).astype(bf16)

    def _pack_w(W, hs):
        # [heads, dh, D] -> [D, HL] (d rows, head-major cols) -> packed
        # [128, 8*HL] so the device DMA is a plain contiguous copy of the
        # on-SBUF layout [p, di, h].
        w = W[hs].transpose(2, 0, 1).reshape(D, HL)
        return np.ascontiguousarray(
            w.reshape(8, 128, HL).transpose(1, 0, 2).reshape(128, 8 * HL)
        ).astype(bf16)

    in_maps = []
    for c in range(N_CORES):
        b, g = c // 4, c % 4
        hs = slice(HPC * g, HPC * g + HPC)
        xT = np.ascontiguousarray(x[b].T).astype(bf16)
        wo = np.ascontiguousarray(W_O[:, HL * g:HL * g + HL].T).astype(bf16)
        in_maps.append(
            {
                "xT": xT,
                "wq": _pack_w(W_Q, hs),
                "wk": _pack_w(W_K, hs),
                "wv": _pack_w(W_V, hs),
                "wo": wo,
                "mask": mask,
                "sel": sel,
            }
        )
    return in_maps


_NC_CACHE = None


def _get_nc():
    global _NC_CACHE
    if _NC_CACHE is None:
        _NC_CACHE = build_nc()
    return _NC_CACHE


def _run(x, W_K, W_Q, W_V, W_O, trace=False):
    nc = _get_nc()
    in_maps = _prep_in_maps(x, W_K, W_Q, W_V, W_O)
    res = run_bass_kernel_spmd(
        nc, in_maps, core_ids=list(range(N_CORES)), trace=trace
    )
    partials = np.stack(
        [np.asarray(res.results[c]["out"]).astype(np.float32) for c in range(N_CORES)]
    )
    out = np.empty((B, S, D), dtype=np.float32)
    out[0] = partials[0:4].sum(axis=0)
    out[1] = partials[4:8].sum(axis=0)
    return out, res


def kernel(x, W_K, W_Q, W_V, W_O):
    out, _ = _run(x, W_K, W_Q, W_V, W_O, trace=False)
    return out


def run_traced(x, W_K, W_Q, W_V, W_O):
    """For test.py: returns (out, BassKernelResults with exec_time_ns)."""
    import types

    if "antenv.axon_hooks" not in sys.modules:
        try:
            from trn_agent_boot.trn_boot import _ntff_profile_via_ctypes

            hook = _ntff_profile_via_ctypes("/opt/axon/libaxon_pjrt.so")
            mod = types.ModuleType("antenv.axon_hooks")
            mod.get_axon_ntff_profile_hook = lambda: hook
            mod.set_axon_ntff_profile_hook = lambda h: None
            sys.modules["antenv.axon_hooks"] = mod
        except Exception:
            pass
    return _run(x, W_K, W_Q, W_V, W_O, trace=True)
